# revision 4
# baseline (speedup 1.0000x reference)
"""Multi-head attention (B=2, S=2048, D=1024, H=16) on 8 TRN2 NeuronCores.

Sharding: core i handles batch b=i//4 and head-group g=i%4 (4 heads of 64 dims
= 256 projection columns per core). Head/batch parallel QKV + attention; the
output projection produces per-core partials summed on the host (no on-device
collectives needed).

Per-core device program (Bass/Tile):
  phase 1: QT/KT = (x Wq)^T, (x Wk)^T in [256, 2048] layout (fp32r matmuls),
           V in natural [2048, 256] layout + a ones column per head (V_aug)
  phase 2 per head:
    sT = K Q^T   [k, q] tiles -> exp (ACT, scale=1/8) -> bf16 resident in SBUF
    ctx'^T + row-sums = V_aug^T @ exp_sT  (ones column yields sum_k exp)
    ctx^T normalized via exp(-ln(rsum)) broadcast (GPSIMD partition_broadcast)
    s  = Q K^T   [q, k] tiles -> exp with accum_out row sums -> reciprocal ->
         tensor_scalar normalize -> DMA straight to attn output (contiguous)
  phase 3: poutT = Wo_g^T @ ctx^T (fp32r) -> DRAM; host sums partials + bias.
"""

import numpy as np

B, S, D, H = 2, 2048, 1024, 16
HD = D // H            # 64
G = 4                  # head-groups (cores per batch)
HPG = H // G           # 4 heads per group
GW = HPG * HD          # 256 group width
N_CORES = 8

_compiled = None


def _build():
    import concourse.bass as bass
    import concourse.tile as tile
    from concourse import bacc, mybir

    F32 = mybir.dt.float32
    F32R = mybir.dt.float32r
    BF16 = mybir.dt.bfloat16
    EXP = mybir.ActivationFunctionType.Exp
    LN = mybir.ActivationFunctionType.Ln

    nc = bacc.Bacc("TRN2", target_bir_lowering=False, debug=False,
                   num_devices=N_CORES)

    xqT = nc.dram_tensor("xqT", [D, S], F32R, kind="ExternalInput").ap()
    xkT = nc.dram_tensor("xkT", [D, S], F32R, kind="ExternalInput").ap()
    xvT = nc.dram_tensor("xvT", [D, S], F32R, kind="ExternalInput").ap()
    wq = nc.dram_tensor("wq", [D, GW], F32R, kind="ExternalInput").ap()
    wk = nc.dram_tensor("wk", [D, GW], F32R, kind="ExternalInput").ap()
    wv = nc.dram_tensor("wv", [D, GW], F32R, kind="ExternalInput").ap()
    wo = nc.dram_tensor("wo", [GW, D], F32R, kind="ExternalInput").ap()
    bq = nc.dram_tensor("bq", [GW], F32, kind="ExternalInput").ap()
    bk = nc.dram_tensor("bk", [GW], F32, kind="ExternalInput").ap()
    bv = nc.dram_tensor("bv", [GW], F32, kind="ExternalInput").ap()
    attn_d = nc.dram_tensor("attn", [HPG, S, S], F32, kind="ExternalOutput").ap()
    poutT_d = nc.dram_tensor("poutT", [D, S], F32, kind="ExternalOutput").ap()

    KT_D = D // 128        # 8 k-tiles over the D contraction
    KT_S = S // 128        # 16 k-tiles over the S contraction
    NQ = S // 512          # 4 512-slices over S

    with tile.TileContext(nc) as tc:
        with tc.tile_pool(name="persist", bufs=1) as persist, \
             tc.tile_pool(name="small", bufs=2) as small, \
             tc.tile_pool(name="acc", bufs=16) as accp, \
             tc.tile_pool(name="attn", bufs=3) as attnp, \
             tc.tile_pool(name="po", bufs=2) as pop, \
             tc.tile_pool(name="pse", bufs=2, space="PSUM") as pse, \
             tc.tile_pool(name="psc", bufs=1, space="PSUM") as psc, \
             tc.tile_pool(name="psp", bufs=2, space="PSUM") as psp:

            # persistent tiles
            qt = persist.tile([128, 2, S], F32R, tag="qt")      # QT [256, S]
            kt_t = persist.tile([128, 2, S], F32R, tag="kt")    # KT [256, S]
            ctxT = persist.tile([128, 2, S], F32R, tag="ctxT")  # ctx^T [256, S]
            v_aug = persist.tile([128, KT_S, HPG, HD + 1], BF16, tag="vaug")
            wo_t = persist.tile([128, 2, D], F32R, tag="wo")
            bq_t = persist.tile([128, 2], F32, tag="bq")
            bk_t = persist.tile([128, 2], F32, tag="bk")
            bv_b = persist.tile([128, GW], F32, tag="bvb")

            nc.sync.dma_start(out=wo_t, in_=wo.rearrange("(t p) n -> p t n", p=128))
            nc.sync.dma_start(out=bq_t, in_=bq.rearrange("(t p) -> p t", p=128))
            nc.sync.dma_start(out=bk_t, in_=bk.rearrange("(t p) -> p t", p=128))
            nc.gpsimd.dma_start(out=bv_b, in_=bv.partition_broadcast(128))
            nc.vector.memset(v_aug[:, :, :, HD], 1.0)

            # ---------------- phase 1: projections ----------------
            with tc.tile_pool(name="wts", bufs=1) as wts, \
                 tc.tile_pool(name="xn", bufs=2) as xnp:
                wq_t = wts.tile([128, KT_D, GW], F32R, tag="wq")
                wk_t = wts.tile([128, KT_D, GW], F32R, tag="wk")
                wv_t = wts.tile([128, KT_D, GW], F32R, tag="wv")
                nc.sync.dma_start(out=wq_t, in_=wq.rearrange("(t p) n -> p t n", p=128))
                nc.sync.dma_start(out=wk_t, in_=wk.rearrange("(t p) n -> p t n", p=128))
                nc.sync.dma_start(out=wv_t, in_=wv.rearrange("(t p) n -> p t n", p=128))

                for n in range(NQ):
                    sl = slice(n * 512, (n + 1) * 512)
                    # QT / KT: out[gw, s] accumulating over D
                    for (xsrc, w_t, b_t, dst) in ((xqT, wq_t, bq_t, qt),
                                                  (xkT, wk_t, bk_t, kt_t)):
                        x_n = xnp.tile([128, KT_D, 512], F32R, tag="xn")
                        nc.sync.dma_start(
                            out=x_n, in_=xsrc[:, sl].rearrange("(t p) s -> p t s", p=128))
                        for mt in range(2):
                            ps = psp.tile([128, 512], F32, tag="proj")
                            for k in range(KT_D):
                                nc.tensor.matmul(
                                    ps, w_t[:, k, mt * 128:(mt + 1) * 128],
                                    x_n[:, k, :], start=(k == 0), stop=(k == KT_D - 1))
                            nc.vector.tensor_scalar_add(
                                dst[:, mt, sl], ps, b_t[:, mt:mt + 1])
                    # V natural layout: out[s, gw] accumulating over D
                    x_n = xnp.tile([128, KT_D, 512], F32R, tag="xn")
                    nc.sync.dma_start(
                        out=x_n, in_=xvT[:, sl].rearrange("(t p) s -> p t s", p=128))
                    for ms in range(4):
                        st = n * 4 + ms
                        ps = psp.tile([128, GW], F32, tag="proj")
                        for k in range(KT_D):
                            nc.tensor.matmul(
                                ps, x_n[:, k, ms * 128:(ms + 1) * 128], wv_t[:, k, :],
                                start=(k == 0), stop=(k == KT_D - 1))
                        nc.vector.tensor_add(
                            v_aug[:, st, :, 0:HD],
                            ps.rearrange("p (h d) -> p h d", h=HPG),
                            bv_b.rearrange("p (h d) -> p h d", h=HPG))

            # ---------------- phase 2: attention per head ----------------
            with tc.tile_pool(name="est", bufs=1) as estp:
                for h in range(HPG):
                    bp = 64 * (h % 2)
                    mt = h // 2
                    q_h = qt[bp:bp + 64, mt, :]
                    k_h = kt_t[bp:bp + 64, mt, :]

                    est = estp.tile([128, KT_S, S], BF16, tag="est")
                    # s^T = K Q^T  [k, q]; exp -> est (bf16)
                    for ktile in range(KT_S):
                        for qh in range(2):
                            ps = pse.tile([128, 1024], F32, tag="sc")
                            for qs in range(2):
                                q0 = qh * 1024 + qs * 512
                                nc.tensor.matmul(
                                    ps[:, qs * 512:(qs + 1) * 512],
                                    k_h[:, ktile * 128:(ktile + 1) * 128],
                                    q_h[:, q0:q0 + 512], start=True, stop=True)
                            nc.scalar.activation(
                                est[:, ktile, qh * 1024:(qh + 1) * 1024], ps,
                                EXP, scale=0.125)
                    # ctx'^T [65, q] = V_aug^T @ exp_sT ; row 64 = rsum^T
                    for qh in range(2):
                        qsl = slice(qh * 1024, (qh + 1) * 1024)
                        pc = psc.tile([65, 1024], F32, tag="ctx")
                        for ktile in range(KT_S):
                            for qs in range(2):
                                nc.tensor.matmul(
                                    pc[:, qs * 512:(qs + 1) * 512],
                                    v_aug[:, ktile, h, :],
                                    est[:, ktile, qh * 1024 + qs * 512:
                                        qh * 1024 + (qs + 1) * 512],
                                    start=(ktile == 0), stop=(ktile == KT_S - 1))
                        lnr = small.tile([1, 1024], F32, tag="lnr")
                        nc.scalar.activation(lnr, pc[64:65, :], LN)
                        rrt = small.tile([1, 1024], F32, tag="rrt")
                        nc.scalar.activation(rrt, lnr, EXP, scale=-1.0)
                        rrb = small.tile([64, 1024], F32, tag="rrb")
                        nc.gpsimd.partition_broadcast(rrb, rrt, channels=64)
                        nc.vector.tensor_mul(ctxT[bp:bp + 64, mt, qsl],
                                             pc[0:64, :], rrb)
                    # s = Q K^T [q, k]; exp + accum row-sums; normalize; DMA out
                    for qt_i in range(KT_S):
                        at = attnp.tile([128, S], F32, tag="attn")
                        acc2 = accp.tile([128, 2], F32, tag="acc2")
                        for kh in range(2):
                            ps = pse.tile([128, 1024], F32, tag="sc")
                            for ks in range(2):
                                k0 = kh * 1024 + ks * 512
                                nc.tensor.matmul(
                                    ps[:, ks * 512:(ks + 1) * 512],
                                    q_h[:, qt_i * 128:(qt_i + 1) * 128],
                                    k_h[:, k0:k0 + 512], start=True, stop=True)
                            nc.scalar.activation(
                                at[:, kh * 1024:(kh + 1) * 1024], ps, EXP,
                                scale=0.125, accum_out=acc2[:, kh:kh + 1])
                        rs = accp.tile([128, 1], F32, tag="rs")
                        nc.vector.tensor_add(rs, acc2[:, 0:1], acc2[:, 1:2])
                        rr = accp.tile([128, 1], F32, tag="rr")
                        nc.vector.reciprocal(rr, rs)
                        nc.vector.tensor_scalar_mul(at, at, rr)
                        nc.sync.dma_start(
                            out=attn_d[h, qt_i * 128:(qt_i + 1) * 128, :], in_=at)

            # ---------------- phase 3: output projection ----------------
            for mt in range(8):
                po = pop.tile([128, S], F32, tag="po")
                for n in range(NQ):
                    ps = psp.tile([128, 512], F32, tag="proj")
                    for k2 in range(2):
                        nc.tensor.matmul(
                            ps, wo_t[:, k2, mt * 128:(mt + 1) * 128],
                            ctxT[:, k2, n * 512:(n + 1) * 512],
                            start=(k2 == 0), stop=(k2 == 1))
                    nc.vector.tensor_scalar_add(po[:, n * 512:(n + 1) * 512], ps, 0.0)
                nc.sync.dma_start(
                    out=poutT_d[mt * 128:(mt + 1) * 128, :], in_=po)

    nc.compile()
    return nc


def _get_compiled():
    global _compiled
    if _compiled is None:
        _compiled = _build()
    return _compiled


def kernel(query, key, value, Wq, bq, Wk, bk, Wv, bv, Wo, bo):
    from concourse import bass_utils

    query = np.asarray(query, dtype=np.float32)
    key = np.asarray(key, dtype=np.float32)
    value = np.asarray(value, dtype=np.float32)
    Wq = np.asarray(Wq, dtype=np.float32)
    Wk = np.asarray(Wk, dtype=np.float32)
    Wv = np.asarray(Wv, dtype=np.float32)
    Wo = np.asarray(Wo, dtype=np.float32)
    bq = np.asarray(bq, dtype=np.float32)
    bk = np.asarray(bk, dtype=np.float32)
    bv = np.asarray(bv, dtype=np.float32)
    bo = np.asarray(bo, dtype=np.float32)

    nc = _get_compiled()

    in_maps = []
    for i in range(N_CORES):
        b, g = divmod(i, G)
        gsl = slice(g * GW, (g + 1) * GW)
        in_maps.append({
            "xqT": np.ascontiguousarray(query[b].T),
            "xkT": np.ascontiguousarray(key[b].T),
            "xvT": np.ascontiguousarray(value[b].T),
            "wq": np.ascontiguousarray(Wq[:, gsl]),
            "wk": np.ascontiguousarray(Wk[:, gsl]),
            "wv": np.ascontiguousarray(Wv[:, gsl]),
            "wo": np.ascontiguousarray(Wo[gsl, :]),
            "bq": np.ascontiguousarray(bq[gsl]),
            "bk": np.ascontiguousarray(bk[gsl]),
            "bv": np.ascontiguousarray(bv[gsl]),
        })

    global _last_in_maps
    _last_in_maps = in_maps
    res = bass_utils.run_bass_kernel_spmd(nc, in_maps, core_ids=list(range(N_CORES)))

    attn = np.empty((B, H, S, S), dtype=np.float32)
    out = np.zeros((B, S, D), dtype=np.float32)
    for i in range(N_CORES):
        b, g = divmod(i, G)
        attn[b, g * HPG:(g + 1) * HPG] = res.results[i]["attn"]
        out[b] += res.results[i]["poutT"].T
    out += bo
    return out, attn


# revision 8
# speedup vs baseline: 1.0537x; 1.0537x over previous
"""Multi-head attention (B=2, S=2048, D=1024, H=16) on 8 TRN2 NeuronCores.

Sharding: core i handles batch b=i//4 and head-group g=i%4 (4 heads of 64 dims
= 256 projection columns per core). Head/batch parallel QKV + attention; the
output projection produces per-core partials summed on the host (no on-device
collectives needed).

Per-core device program (Bass/Tile):
  phase 1: QT/KT = (x Wq)^T, (x Wk)^T in [256, 2048] layout (fp32r matmuls),
           V in natural [2048, 256] layout + a ones column per head (V_aug)
  phase 2 per head:
    sT = K Q^T   [k, q] tiles -> exp (ACT, scale=1/8) -> bf16 resident in SBUF
    ctx'^T + row-sums = V_aug^T @ exp_sT  (ones column yields sum_k exp)
    ctx^T normalized via exp(-ln(rsum)) broadcast (GPSIMD partition_broadcast)
    s  = Q K^T   [q, k] tiles -> exp with accum_out row sums -> reciprocal ->
         tensor_scalar normalize -> DMA straight to attn output (contiguous)
  phase 3: poutT = Wo_g^T @ ctx^T (fp32r) -> DRAM; host sums partials + bias.
"""

import numpy as np

B, S, D, H = 2, 2048, 1024, 16
HD = D // H            # 64
G = 4                  # head-groups (cores per batch)
HPG = H // G           # 4 heads per group
GW = HPG * HD          # 256 group width
N_CORES = 8

_compiled = None


def _build():
    import concourse.bass as bass
    import concourse.tile as tile
    from concourse import bacc, mybir

    F32 = mybir.dt.float32
    F32R = mybir.dt.float32r
    BF16 = mybir.dt.bfloat16
    EXP = mybir.ActivationFunctionType.Exp

    nc = bacc.Bacc("TRN2", target_bir_lowering=False, debug=False,
                   num_devices=N_CORES)

    xqT = nc.dram_tensor("xqT", [D, S], F32R, kind="ExternalInput").ap()
    xkT = nc.dram_tensor("xkT", [D, S], F32R, kind="ExternalInput").ap()
    xvT = nc.dram_tensor("xvT", [D, S], F32R, kind="ExternalInput").ap()
    wq = nc.dram_tensor("wq", [D, GW], F32R, kind="ExternalInput").ap()
    wk = nc.dram_tensor("wk", [D, GW], F32R, kind="ExternalInput").ap()
    wv = nc.dram_tensor("wv", [D, GW], F32R, kind="ExternalInput").ap()
    wo = nc.dram_tensor("wo", [GW, D], F32R, kind="ExternalInput").ap()
    bq = nc.dram_tensor("bq", [GW], F32, kind="ExternalInput").ap()
    bk = nc.dram_tensor("bk", [GW], F32, kind="ExternalInput").ap()
    bv = nc.dram_tensor("bv", [GW], F32, kind="ExternalInput").ap()
    attn_d = nc.dram_tensor("attn", [HPG, S, S], F32, kind="ExternalOutput").ap()
    poutT_d = nc.dram_tensor("poutT", [D, S], F32, kind="ExternalOutput").ap()

    KT_D = D // 128        # 8 k-tiles over the D contraction
    KT_S = S // 128        # 16 k-tiles over the S contraction
    NQ = S // 512          # 4 512-slices over S

    with tile.TileContext(nc) as tc:
        with tc.tile_pool(name="persist", bufs=1) as persist, \
             tc.tile_pool(name="small", bufs=2) as small, \
             tc.tile_pool(name="acc", bufs=16) as accp, \
             tc.tile_pool(name="attn", bufs=3) as attnp, \
             tc.tile_pool(name="po", bufs=2) as pop, \
             tc.tile_pool(name="pse", bufs=2, space="PSUM") as pse, \
             tc.tile_pool(name="psc", bufs=2, space="PSUM") as psc:

            # persistent tiles
            qt = persist.tile([128, 2, S], F32R, tag="qt")      # QT [256, S]
            kt_t = persist.tile([128, 2, S], F32R, tag="kt")    # KT [256, S]
            ctxT = persist.tile([128, 2, S], F32R, tag="ctxT")  # ctx^T [256, S]
            v_aug = persist.tile([128, KT_S, HPG, HD + 1], BF16, tag="vaug")
            wo_t = persist.tile([128, 2, D], F32R, tag="wo")
            bq_t = persist.tile([128, 2], F32, tag="bq")
            bk_t = persist.tile([128, 2], F32, tag="bk")
            bv_b = persist.tile([128, GW], F32, tag="bvb")

            nc.sync.dma_start(out=wo_t, in_=wo.rearrange("(t p) n -> p t n", p=128))
            nc.sync.dma_start(out=bq_t, in_=bq.rearrange("(t p) -> p t", p=128))
            nc.sync.dma_start(out=bk_t, in_=bk.rearrange("(t p) -> p t", p=128))
            nc.gpsimd.dma_start(out=bv_b, in_=bv.partition_broadcast(128))
            nc.vector.memset(v_aug[:, :, :, HD], 1.0)

            # ---------------- phase 1: projections ----------------
            with tc.tile_pool(name="wts", bufs=1) as wts, \
                 tc.tile_pool(name="xn", bufs=3) as xnp:
                wq_t = wts.tile([128, KT_D, GW], F32R, tag="wq")
                wk_t = wts.tile([128, KT_D, GW], F32R, tag="wk")
                wv_t = wts.tile([128, KT_D, GW], F32R, tag="wv")
                nc.sync.dma_start(out=wq_t, in_=wq.rearrange("(t p) n -> p t n", p=128))
                nc.sync.dma_start(out=wk_t, in_=wk.rearrange("(t p) n -> p t n", p=128))
                nc.sync.dma_start(out=wv_t, in_=wv.rearrange("(t p) n -> p t n", p=128))

                for n in range(NQ):
                    sl = slice(n * 512, (n + 1) * 512)
                    # QT / KT: out[gw, s] accumulating over D
                    for (xsrc, w_t, b_t, dst) in ((xqT, wq_t, bq_t, qt),
                                                  (xkT, wk_t, bk_t, kt_t)):
                        x_n = xnp.tile([128, KT_D, 512], F32R, tag="xn")
                        nc.sync.dma_start(
                            out=x_n, in_=xsrc[:, sl].rearrange("(t p) s -> p t s", p=128))
                        for mt in range(2):
                            ps = pse.tile([128, 1024], F32, tag="sc")
                            for k in range(KT_D):
                                nc.tensor.matmul(
                                    ps[:, 0:512], w_t[:, k, mt * 128:(mt + 1) * 128],
                                    x_n[:, k, :], start=(k == 0), stop=(k == KT_D - 1))
                            nc.vector.tensor_scalar_add(
                                dst[:, mt, sl], ps[:, 0:512], b_t[:, mt:mt + 1])
                    # V natural layout: out[s, gw] accumulating over D
                    x_n = xnp.tile([128, KT_D, 512], F32R, tag="xn")
                    nc.sync.dma_start(
                        out=x_n, in_=xvT[:, sl].rearrange("(t p) s -> p t s", p=128))
                    for ms in range(4):
                        st = n * 4 + ms
                        ps = pse.tile([128, 1024], F32, tag="sc")
                        for k in range(KT_D):
                            nc.tensor.matmul(
                                ps[:, 0:GW], x_n[:, k, ms * 128:(ms + 1) * 128],
                                wv_t[:, k, :], start=(k == 0), stop=(k == KT_D - 1))
                        nc.vector.tensor_add(
                            v_aug[:, st, :, 0:HD],
                            ps[:, 0:GW].rearrange("p (h d) -> p h d", h=HPG),
                            bv_b.rearrange("p (h d) -> p h d", h=HPG))

            # ---------------- phase 2: attention per (head, q-half) ----------------
            with tc.tile_pool(name="est", bufs=2) as estp:
                for h in range(HPG):
                    bp = 64 * (h % 2)
                    mt = h // 2
                    q_h = qt[bp:bp + 64, mt, :]
                    k_h = kt_t[bp:bp + 64, mt, :]

                    rrec8s = []
                    for qh in range(2):
                        q0 = qh * 1024
                        # s^T = K Q^T [k, q-half]; exp -> est (bf16)
                        est = estp.tile([128, KT_S, 1024], BF16, tag="est")
                        for ktile in range(KT_S):
                            ps = pse.tile([128, 1024], F32, tag="sc")
                            for qs in range(2):
                                nc.tensor.matmul(
                                    ps[:, qs * 512:(qs + 1) * 512],
                                    k_h[:, ktile * 128:(ktile + 1) * 128],
                                    q_h[:, q0 + qs * 512:q0 + (qs + 1) * 512],
                                    start=True, stop=True)
                            nc.scalar.activation(est[:, ktile, :], ps, EXP, scale=0.125)
                        # ctx'^T [65, q-half]; row 64 = rsum^T
                        pc = psc.tile([65, 1024], F32, tag="ctx")
                        for ktile in range(KT_S):
                            for qs in range(2):
                                nc.tensor.matmul(
                                    pc[:, qs * 512:(qs + 1) * 512],
                                    v_aug[:, ktile, h, :],
                                    est[:, ktile, qs * 512:(qs + 1) * 512],
                                    start=(ktile == 0), stop=(ktile == KT_S - 1))
                        rcp = small.tile([1, 1024], F32, tag="rcp")
                        nc.vector.tensor_copy(rcp, pc[64:65, :])
                        rrt = small.tile([1, 1024], F32, tag="rrt")
                        nc.vector.reciprocal_approx_fast(rrt, rcp)
                        rrb = small.tile([64, 1024], F32, tag="rrb")
                        nc.gpsimd.partition_broadcast(rrb, rrt, channels=64)
                        nc.vector.tensor_mul(ctxT[bp:bp + 64, mt, q0:q0 + 1024],
                                             pc[0:64, :], rrb)
                    # s = Q K^T [q, k]; exp + accum row-sums; normalize; DMA out
                    for qt_i in range(KT_S):
                        at = attnp.tile([128, S], F32, tag="attn")
                        acc2 = accp.tile([128, 2], F32, tag="acc2")
                        for kh in range(2):
                            ps = pse.tile([128, 1024], F32, tag="sc")
                            for ks in range(2):
                                k0 = kh * 1024 + ks * 512
                                nc.tensor.matmul(
                                    ps[:, ks * 512:(ks + 1) * 512],
                                    q_h[:, qt_i * 128:(qt_i + 1) * 128],
                                    k_h[:, k0:k0 + 512], start=True, stop=True)
                            nc.scalar.activation(
                                at[:, kh * 1024:(kh + 1) * 1024], ps, EXP,
                                scale=0.125, accum_out=acc2[:, kh:kh + 1])
                        rs = accp.tile([128, 1], F32, tag="rs")
                        nc.vector.tensor_add(rs, acc2[:, 0:1], acc2[:, 1:2])
                        rr = accp.tile([128, 1], F32, tag="rr")
                        nc.vector.reciprocal(rr, rs)
                        nc.vector.tensor_scalar_mul(at, at, rr)
                        eng = nc.sync if qt_i % 2 == 0 else nc.gpsimd
                        eng.dma_start(
                            out=attn_d[h, qt_i * 128:(qt_i + 1) * 128, :], in_=at)

            # ---------------- phase 3: output projection ----------------
            for mt in range(8):
                po = pop.tile([128, S], F32, tag="po")
                for n in range(NQ):
                    ps = psc.tile([128, 512], F32, tag="ctx")
                    for k2 in range(2):
                        nc.tensor.matmul(
                            ps, wo_t[:, k2, mt * 128:(mt + 1) * 128],
                            ctxT[:, k2, n * 512:(n + 1) * 512],
                            start=(k2 == 0), stop=(k2 == 1))
                    nc.vector.tensor_scalar_add(po[:, n * 512:(n + 1) * 512], ps, 0.0)
                eng = nc.sync if mt % 2 == 0 else nc.gpsimd
                eng.dma_start(out=poutT_d[mt * 128:(mt + 1) * 128, :], in_=po)

    nc.compile()
    return nc


def _get_compiled():
    global _compiled
    if _compiled is None:
        _compiled = _build()
    return _compiled


def kernel(query, key, value, Wq, bq, Wk, bk, Wv, bv, Wo, bo):
    from concourse import bass_utils

    query = np.asarray(query, dtype=np.float32)
    key = np.asarray(key, dtype=np.float32)
    value = np.asarray(value, dtype=np.float32)
    Wq = np.asarray(Wq, dtype=np.float32)
    Wk = np.asarray(Wk, dtype=np.float32)
    Wv = np.asarray(Wv, dtype=np.float32)
    Wo = np.asarray(Wo, dtype=np.float32)
    bq = np.asarray(bq, dtype=np.float32)
    bk = np.asarray(bk, dtype=np.float32)
    bv = np.asarray(bv, dtype=np.float32)
    bo = np.asarray(bo, dtype=np.float32)

    nc = _get_compiled()

    in_maps = []
    for i in range(N_CORES):
        b, g = divmod(i, G)
        gsl = slice(g * GW, (g + 1) * GW)
        in_maps.append({
            "xqT": np.ascontiguousarray(query[b].T),
            "xkT": np.ascontiguousarray(key[b].T),
            "xvT": np.ascontiguousarray(value[b].T),
            "wq": np.ascontiguousarray(Wq[:, gsl]),
            "wk": np.ascontiguousarray(Wk[:, gsl]),
            "wv": np.ascontiguousarray(Wv[:, gsl]),
            "wo": np.ascontiguousarray(Wo[gsl, :]),
            "bq": np.ascontiguousarray(bq[gsl]),
            "bk": np.ascontiguousarray(bk[gsl]),
            "bv": np.ascontiguousarray(bv[gsl]),
        })

    global _last_in_maps
    _last_in_maps = in_maps
    res = bass_utils.run_bass_kernel_spmd(nc, in_maps, core_ids=list(range(N_CORES)))

    attn = np.empty((B, H, S, S), dtype=np.float32)
    out = np.zeros((B, S, D), dtype=np.float32)
    for i in range(N_CORES):
        b, g = divmod(i, G)
        attn[b, g * HPG:(g + 1) * HPG] = res.results[i]["attn"]
        out[b] += res.results[i]["poutT"].T
    out += bo
    return out, attn
